# revision 9
# baseline (speedup 1.0000x reference)
"""Trainium2 Bass kernel for the dynamic segment-aggregation module (v3).

Computation per (clip n, channel c):
  pooled[u]  = mean_{t,h,w} x[n,c,u,...]                (U=4 segments)
  z          = relu(BN(pooled @ W1^T))                  (tiny MLP, eval-mode BN)
  kern       = softmax(z @ W2^T)                        (K=3 taps)
  out[u]     = kern[0]*x[u-1] + kern[1]*x[u] + kern[2]*x[u+1]   (zero-padded)

Sharding: data-parallel over the 8 clips -> 1 clip (4 U-segments) per
NeuronCore; generator weights replicated.

v3 strategy (fp16 HBM + TensorE blend + distributed pooling):
  - fp16 device I/O halves HBM traffic to 25.7 MB/core (~72 us floor).
  - Blend rows u1..u3 run on TensorE: matmul with a DIAGONAL stationary
    diag(k_j[c]) is a per-channel scalar multiply and PSUM accumulation
    does the tap adds (8 taps).  Row u0 (2 taps) runs on DVE at 4x/2x
    perf modes so the PE chain and the PSUM-drain load both shrink.
  - PSUM is tiled [128, 3, 512] (3 banks, one per u-row); one drain op
    copies all 3 rows -> fewer 1x PSUM reads.  Drains split DVE/ACT.
  - Pooling is spread over three engines so nothing stalls the drains:
    DVE uses a 2x tensor_tensor halving tree (accum_out runs 1x only),
    ACT uses activation accumulators, GpSimd (otherwise idle) uses its
    reduce_sum for two of the group-1 half-slabs.
  - Stores ride the ACT HWDGE queue (GpSimd descriptor-gen time is spent
    pooling); loads stay on the Sync HWDGE queue.
"""

import numpy as np

import concourse.bass as bass
import concourse.bacc as bacc
import concourse.tile as tile
from concourse import mybir
from concourse.tile_rust import add_dep_helper
from concourse.bass_utils import run_bass_kernel_spmd

U = 4          # segments per clip
C = 256        # channels
T, H, W = 8, 28, 28
THW = T * H * W            # 6272
NS = 2                     # f-slabs per channel group
FS = THW // NS             # 3136
FH = FS // 2               # 1568: half-slab load/pool granularity
CK = 512                   # PSUM chunk (fp32 elems = one 2KB bank)
CHUNKS = [(i * CK, min((i + 1) * CK, FS)) for i in range((FS + CK - 1) // CK)]
D = 8                      # MLP hidden dim (U * alpha)
K = 3                      # conv taps
EPS = 1e-5
N_CORES = 8

# packed small-weights layout: [W1*(1/THW) (D*U) | W2 (K*D) | s (D) | t (D)]
NPACK = D * U + K * D + D + D    # 72

FP32 = mybir.dt.float32
F16 = mybir.dt.float16

# PE tap table per output row u in 1..3: (j, usrc, start, stop) with
# out[u] += k_j * x[u-1+j].  Row 0 (k1*x0 + k2*x1) runs on DVE.
PE_TAPS = {
    1: [(0, 0, True, False), (1, 1, False, False), (2, 2, False, True)],
    2: [(0, 1, True, False), (1, 2, False, False), (2, 3, False, True)],
    3: [(0, 2, True, False), (1, 3, False, True)],
}

_nc_cache = None
last_results = None        # BassKernelResults of the most recent run (for test.py)


def _bcast_ap(ap, parts=128):
    """DRAM AP replicated across `parts` partitions (partition stride 0)."""
    return bass.AP(tensor=ap.tensor, offset=ap.offset, ap=[[0, parts]] + list(ap.ap))


def _build_nc():
    nc = bacc.Bacc(None, target_bir_lowering=False)
    x_h = nc.declare_dram_parameter("x", [U, C, THW], F16, isOutput=False)
    wp_h = nc.declare_dram_parameter("wpack", [NPACK], FP32, isOutput=False)
    id_h = nc.declare_dram_parameter("ident", [128, 128], F16, isOutput=False)
    out_h = nc.declare_dram_parameter("out", [U, C, THW], F16, isOutput=True)

    xg = x_h[:].rearrange("u c f -> c u f")      # [C, U, THW]
    og = out_h[:].rearrange("u c f -> c u f")

    AX = mybir.AxisListType
    OP = mybir.AluOpType
    AF = mybir.ActivationFunctionType

    with tile.TileContext(nc) as tc:
        with (
            tc.tile_pool(name="xp", bufs=4) as xp,
            tc.tile_pool(name="outp", bufs=2) as outp,
            tc.tile_pool(name="t0p", bufs=2) as t0p,
            tc.tile_pool(name="treep", bufs=2) as treep,
            tc.tile_pool(name="small", bufs=1) as small,
            tc.tile_pool(name="mlp", bufs=2) as mlp,
            tc.tile_pool(name="wp", bufs=2) as wp,
            tc.psum_pool(name="pp", bufs=2) as pp,
        ):
            # per-core-replicated constants
            wpk = small.tile([128, NPACK], FP32, tag="wpk")
            nc.gpsimd.dma_start(out=wpk, in_=_bcast_ap(wp_h[:]))
            ident = small.tile([128, 128], F16, tag="ident")
            nc.sync.dma_start(out=ident, in_=id_h[:])
            w1sb = wpk[:, 0:D * U].rearrange("p (d u) -> p d u", d=D)       # [128,D,U]
            w2sb = wpk[:, D * U:D * U + K * D].rearrange(
                "p (k d) -> p k d", k=K)                                    # [128,K,D]
            s_t = wpk[:, D * U + K * D:D * U + K * D + D]                   # [128,D]
            o_t = wpk[:, D * U + K * D + D:NPACK]                           # [128,D]

            # warm the ACT Exp table set while the first loads stream
            warm = small.tile([128, 1], FP32, tag="warm")
            nc.scalar.activation(out=warm, in_=wpk[:, 0:1], func=AF.Exp)

            def load_slab(g, s):
                c0, f0 = g * 128, s * FS
                sl = xp.tile([128, U, FS], F16, tag="slab", name=f"sl{g}{s}")
                for h, eng in ((0, nc.sync), (1, nc.scalar)):
                    eng.dma_start(
                        out=sl[:, :, h * FH:(h + 1) * FH],
                        in_=xg[c0:c0 + 128, :, f0 + h * FH:f0 + h * FH + FH],
                    )
                return sl

            def tree_pool_half(sl, h, P, idx, rows, after=None):
                """DVE halving tree: tensor_tensor runs 2x on fp16 while
                accum_out ops are stuck at 1x; 3 halvings then a short
                1x accumulate per row."""
                b = h * FH
                nr = len(rows)
                r0 = rows[0]
                y1 = treep.tile([128, U, FH // 2], F16, tag="y1", name="y1")
                r1 = nc.vector.tensor_add(
                    out=y1[:, 0:nr, :],
                    in0=sl[:, r0:r0 + nr, b:b + FH // 2],
                    in1=sl[:, r0:r0 + nr, b + FH // 2:b + FH],
                )
                if after is not None:
                    add_dep_helper(r1.ins, after.ins,
                                   reason="pool tree yields DVE to MLP chain")
                y2 = treep.tile([128, U, FH // 4], F16, tag="y2", name="y2")
                nc.vector.tensor_add(
                    out=y2[:, 0:nr, :],
                    in0=y1[:, 0:nr, 0:FH // 4],
                    in1=y1[:, 0:nr, FH // 4:FH // 2],
                )
                nc.vector.tensor_add(
                    out=y1[:, 0:nr, 0:FH // 8],
                    in0=y2[:, 0:nr, 0:FH // 8],
                    in1=y2[:, 0:nr, FH // 8:FH // 4],
                )
                for i, u in enumerate(rows):
                    nc.vector.tensor_scalar(
                        out=y2[:, i, 0:FH // 8], in0=y1[:, i, 0:FH // 8],
                        scalar1=1.0, scalar2=0.0, op0=OP.mult, op1=OP.add,
                        accum_out=P[:, u, idx:idx + 1],
                    )

            def act_pool_rows(sl, h, P, idx, rows):
                hs = slice(h * FH, (h + 1) * FH)
                for u in rows:
                    nc.scalar.activation(
                        out=sl[:, u, hs], in_=sl[:, u, hs], func=AF.Copy,
                        accum_out=P[:, u, idx:idx + 1],
                    )

            def gen_mlp(P):
                """pooled -> relu(BN(pooled@W1^T)) -> softmax(z@W2^T)."""
                pooled = mlp.tile([128, U], FP32, tag="pooled")
                nc.vector.reduce_sum(out=pooled, in_=P, axis=AX.X)
                z = mlp.tile([128, D], FP32, tag="z")
                nc.vector.tensor_scalar_mul(
                    out=z, in0=w1sb[:, :, 0], scalar1=pooled[:, 0:1]
                )
                for u in range(1, U):
                    nc.vector.scalar_tensor_tensor(
                        out=z, in0=w1sb[:, :, u], scalar=pooled[:, u:u + 1],
                        in1=z, op0=OP.mult, op1=OP.add,
                    )
                nc.vector.tensor_mul(out=z, in0=z, in1=s_t)
                nc.vector.tensor_add(out=z, in0=z, in1=o_t)
                nc.vector.tensor_scalar_max(out=z, in0=z, scalar1=0.0)
                logit = mlp.tile([128, K], FP32, tag="logit")
                nc.vector.tensor_scalar_mul(
                    out=logit, in0=w2sb[:, :, 0], scalar1=z[:, 0:1]
                )
                for d in range(1, D):
                    nc.vector.scalar_tensor_tensor(
                        out=logit, in0=w2sb[:, :, d], scalar=z[:, d:d + 1],
                        in1=logit, op0=OP.mult, op1=OP.add,
                    )
                mx = mlp.tile([128, 1], FP32, tag="mx")
                nc.vector.reduce_max(out=mx, in_=logit, axis=AX.X)
                nc.vector.tensor_scalar_mul(out=mx, in0=mx, scalar1=-1.0)
                nc.scalar.activation(
                    out=logit, in_=logit, func=AF.Exp, bias=mx[:, 0:1]
                )
                ssum = mlp.tile([128, 1], FP32, tag="ssum")
                nc.vector.reduce_sum(out=ssum, in_=logit, axis=AX.X)
                nc.vector.reciprocal(out=ssum, in_=ssum)
                kern = mlp.tile([128, K], FP32, tag="kern")
                nc.vector.tensor_scalar_mul(out=kern, in0=logit, scalar1=ssum[:, 0:1])
                return kern

            def make_w(kern):
                """Diagonal stationary matrices diag(k_j[c]), fp16."""
                w = wp.tile([128, K, 128], F16, tag="W", name="w")
                last = None
                for j in range(K):
                    last = nc.vector.tensor_scalar_mul(
                        out=w[:, j, :], in0=ident, scalar1=kern[:, j:j + 1]
                    )
                return w, last

            def blend_slab(g, s, sl, kern, w, dve_drains=(0, 1), mid=None):
                """u0 on DVE (TS 4x + in-place TT 2x); u1-u3 on PE via the
                diagonal matmuls; 3-row batched PSUM drains; stores on the
                ACT HWDGE queue at ~1MB granularity."""
                c0, f0 = g * 128, s * FS
                out_sb = outp.tile([128, U, FS], F16, tag="outsb", name="osb")
                t0 = t0p.tile([128, FS], F16, tag="t0", name="t0")
                nc.vector.tensor_scalar_mul(out=t0, in0=sl[:, 1, :],
                                            scalar1=kern[:, 2:3])
                nc.vector.tensor_scalar_mul(out=out_sb[:, 0, :], in0=sl[:, 0, :],
                                            scalar1=kern[:, 1:2])
                nc.vector.tensor_add(out=out_sb[:, 0, :], in0=out_sb[:, 0, :],
                                     in1=t0)
                stored = 0
                for ci, (a, b) in enumerate(CHUNKS):
                    n = b - a
                    pt = pp.tile([128, 3, CK], FP32, tag="pt", name=f"pt{ci}")
                    for j in range(K):
                        for u in (1, 2, 3):
                            for (tj, usrc, st, sp) in PE_TAPS[u]:
                                if tj == j:
                                    nc.tensor.matmul(
                                        pt[:, u - 1, 0:n], w[:, j, :],
                                        sl[:, usrc, a:b], start=st, stop=sp,
                                    )
                    if ci in dve_drains:
                        nc.vector.tensor_copy(out_sb[:, 1:4, a:b], pt[:, :, 0:n])
                    else:
                        nc.scalar.activation(
                            out=out_sb[:, 1:4, a:b], in_=pt[:, :, 0:n],
                            func=AF.Copy,
                        )
                    if mid is not None and ci == 1:
                        mid()
                        mid = None
                    # store after chunks 1, 3 and the final chunk
                    if ci in (1, 3, len(CHUNKS) - 1):
                        nc.gpsimd.dma_start(
                            out=og[c0:c0 + 128, :, f0 + stored:f0 + b],
                            in_=out_sb[:, :, stored:b],
                        )
                        stored = b

            # ---- program ----
            slabs = {(g, s): load_slab(g, s) for g in (0, 1) for s in range(NS)}

            P0 = mlp.tile([128, U, 2 * NS], FP32, tag="P", name="P0")
            P1 = mlp.tile([128, U, 2 * NS], FP32, tag="P", name="P1")

            # group-0 pooling: DVE trees during the load window; the last
            # half splits DVE/ACT so both finish at load-completion time
            tree_pool_half(slabs[(0, 0)], 0, P0, 0, rows=(0, 1, 2, 3))
            tree_pool_half(slabs[(0, 0)], 1, P0, 1, rows=(0, 1, 2, 3))
            tree_pool_half(slabs[(0, 1)], 0, P0, 2, rows=(0, 1, 2, 3))
            tree_pool_half(slabs[(0, 1)], 1, P0, 3, rows=(0, 1))
            act_pool_rows(slabs[(0, 1)], 1, P0, 3, rows=(2, 3))

            kern0 = gen_mlp(P0)
            w0, w0_last = make_w(kern0)

            def mid_g0():
                tree_pool_half(slabs[(1, 0)], 0, P1, 0, rows=(0, 1, 2, 3),
                               after=w0_last)
                tree_pool_half(slabs[(1, 0)], 1, P1, 1, rows=(0, 1, 2, 3),
                               after=w0_last)

            blend_slab(0, 0, slabs[(0, 0)], kern0, w0, dve_drains=(0,),
                       mid=mid_g0)

            def mid_g1():
                tree_pool_half(slabs[(1, 1)], 0, P1, 2, rows=(0, 1, 2, 3),
                               after=w0_last)
                tree_pool_half(slabs[(1, 1)], 1, P1, 3, rows=(0, 1),
                               after=w0_last)
                act_pool_rows(slabs[(1, 1)], 1, P1, 3, rows=(2, 3))
                global_kern1[0] = gen_mlp(P1)
                global_w1[0], _ = make_w(global_kern1[0])

            global_kern1 = [None]
            global_w1 = [None]
            blend_slab(0, 1, slabs[(0, 1)], kern0, w0, dve_drains=(0,),
                       mid=mid_g1)
            blend_slab(1, 0, slabs[(1, 0)], global_kern1[0], global_w1[0])
            blend_slab(1, 1, slabs[(1, 1)], global_kern1[0], global_w1[0])
    nc.finalize()
    return nc


def _get_nc():
    global _nc_cache
    if _nc_cache is None:
        _nc_cache = _build_nc()
    return _nc_cache


def _pack_small(W1, bn_gamma, bn_beta, bn_mean, bn_var, W2):
    W1 = np.asarray(W1, np.float32)
    W2 = np.asarray(W2, np.float32)
    gam = np.asarray(bn_gamma, np.float32)
    bet = np.asarray(bn_beta, np.float32)
    mea = np.asarray(bn_mean, np.float32)
    var = np.asarray(bn_var, np.float32)
    s = (gam / np.sqrt(var + np.float32(EPS))).astype(np.float32)
    t = (bet - mea * s).astype(np.float32)
    w1s = (W1 * np.float32(1.0 / THW)).astype(np.float32)
    return np.concatenate(
        [w1s.reshape(-1), W2.reshape(-1), s, t]
    ).astype(np.float32)


def _ensure_hook_stub():
    """bass_utils' trace path imports antenv.axon_hooks when BASS_TRACE is
    set; if this image lacks it, register a None-returning stub so the run
    degrades to no-trace instead of crashing."""
    import sys
    import types

    try:
        import antenv.axon_hooks  # noqa: F401
    except ImportError:
        mod = types.ModuleType("antenv.axon_hooks")
        mod.get_axon_ntff_profile_hook = lambda: None
        mod.set_axon_ntff_profile_hook = lambda h: None
        sys.modules["antenv.axon_hooks"] = mod


def kernel(x, W1, bn_gamma, bn_beta, bn_mean, bn_var, W2):
    global last_results
    _ensure_hook_stub()
    nc = _get_nc()
    x16 = np.ascontiguousarray(
        np.asarray(x, dtype=np.float32).astype(np.float16)
    ).reshape(N_CORES, U, C, THW)
    wpack = _pack_small(W1, bn_gamma, bn_beta, bn_mean, bn_var, W2)
    ident = np.eye(128, dtype=np.float16)
    in_maps = [
        {"x": x16[i], "wpack": wpack, "ident": ident} for i in range(N_CORES)
    ]
    last_results = run_bass_kernel_spmd(nc, in_maps, list(range(N_CORES)))
    out = np.stack([last_results.results[i]["out"] for i in range(N_CORES)])
    return out.reshape(N_CORES * U, C, T, H, W).astype(np.float32)
